# revision 52
# baseline (speedup 1.0000x reference)
"""GroupQueryAttention (16 heads, hd=128) on 8 trn2 cores, heads sharded 2/core.

v3: bf16 data path + wide-exp attention inner loop.
- Scores accumulate into [128,1024] 2-bank PSUM tiles (two k-tiles per tile);
  one FD=1024 ACT exp per pair halves ACT instruction overhead (ACT is the
  binding engine during attention).
- PSUM (8 banks): sc 2x[128,1024] (4) + pv 2 + rb 1 + op 1. q/k projection
  pair-chains and chunk-0 chains borrow the sc banks; v chains use pv;
  out-proj blocks rotate op/rb (final chunk: op/pv/sc). One accumulation
  chain per bank ONLY - matmul start=True clears the whole bank on HW.
- q2/k2 hold both heads side by side so each projection pair-chain retires
  with a single wide psum->sbuf copy.
- Out-proj stages 4 oc-blocks into one [128,2048] ob tile, one store DMA per
  128-token tile (sync/SP queue); x-chunk prefetches ride the gpsimd queue.

Layouts (per core c, host-prepped):
  xT    [B, 4, 16, 128, 512] bf16  x[b].T chunked: (chunk, ktile, h-part, t-col)
  wqT/wkT/wvT [16, 128, 256] bf16  W[256c:256c+256,:].T chunked by h-ktile
  woT   [2, 128, 2048] bf16        Wo[:, 256c:256c+256].T per local head
  out   [4096, 2048] bf16          partial product, host sums over cores

Device per (b, h): scoresT[tk,tq] = kT.T@qT pairs -> exp FD=1024 (ACT,
psum->sbuf bf16) -> PV chain attnT[hd,tq]; rowsum via ones-col matmul over a
DVE bf16 add-tree; normalize attnT via PE-broadcast reciprocal; out-proj from
attnT (bf16 stationary) @ woT, deferred past the batch boundary for overlap.
"""
import sys

for _p in ("/opt/trn_rl_repo",):
    if _p not in sys.path:
        sys.path.insert(0, _p)

import numpy as np
import ml_dtypes

import concourse.bass as bass
import concourse.tile as tile
from concourse import bacc, mybir
from concourse.bass_utils import run_bass_kernel_spmd

N_CORES = 8
B, T, H = 2, 2048, 2048
NH, HD = 16, 128
HPC = H // N_CORES          # 256 dims (2 heads) per core
HEADS_PC = NH // N_CORES    # 2
KT = H // 128               # 16 k-tiles along hidden
TCH = 4                     # t-chunks (512 cols) per batch for projections
TC = T // TCH               # 512
QC = 512                    # tq chunk in attention
NQC = T // QC               # 4
NJ = KT // 2                # 8 score-pairs (big tiles) per head-chunk
SCALE = float(HD) ** -0.5
XT_SZ = B * TCH * KT * 128 * TC      # hidden_states.T, chunked
W_SZ = KT * 128 * HPC                # one projection weight slice
BLOB_SZ = XT_SZ + 4 * W_SZ

F32 = mybir.dt.float32
F32R = mybir.dt.float32r
BF16 = mybir.dt.bfloat16
AF = mybir.ActivationFunctionType
OP = mybir.AluOpType
NPBF16 = ml_dtypes.bfloat16


def r(ap):
    return ap


_CACHE = {}


def _build(use_mask, use_bias):
    key = (use_mask, use_bias)
    if key in _CACHE:
        return _CACHE[key]

    nc = bacc.Bacc("TRN2", target_bir_lowering=False, debug=False,
                   num_devices=N_CORES)
    # all inputs packed into one blob: per-call dispatch cost through the
    # axon tunnel scales with operand count, so one buffer beats six
    blob = nc.dram_tensor("blob", [BLOB_SZ], BF16, kind="ExternalInput").ap()
    xT = blob[0:XT_SZ].rearrange("(b c i p j) -> b c i p j",
                                 b=B, c=TCH, i=KT, p=128, j=TC)
    _o = XT_SZ
    wqT = blob[_o:_o + W_SZ].rearrange("(i p j) -> i p j", i=KT, p=128, j=HPC)
    _o += W_SZ
    wkT = blob[_o:_o + W_SZ].rearrange("(i p j) -> i p j", i=KT, p=128, j=HPC)
    _o += W_SZ
    wvT = blob[_o:_o + W_SZ].rearrange("(i p j) -> i p j", i=KT, p=128, j=HPC)
    _o += W_SZ
    woT = blob[_o:_o + W_SZ].rearrange("(h p j) -> h p j", h=HEADS_PC, p=128, j=H)
    if use_bias:
        bqd = nc.dram_tensor("bq", [HEADS_PC, 128], F32, kind="ExternalInput").ap()
        bkd = nc.dram_tensor("bk", [HEADS_PC, 128], F32, kind="ExternalInput").ap()
        bvd = nc.dram_tensor("bv", [1, HPC], F32R, kind="ExternalInput").ap()
    if use_mask:
        # mask[b,0].T / SCALE, tk-tiled
        mkd = nc.dram_tensor("maskT", [B, KT, 128, T], F32, kind="ExternalInput").ap()
    out = nc.dram_tensor("out", [B * T, H], BF16, kind="ExternalOutput").ap()

    from contextlib import ExitStack
    with tile.TileContext(nc) as tc, ExitStack() as ctx:
        wpool = ctx.enter_context(tc.tile_pool(name="wts", bufs=1))
        cpool = ctx.enter_context(tc.tile_pool(name="consts", bufs=1))
        xpool = ctx.enter_context(tc.tile_pool(name="xt", bufs=4))
        qkv_pool = ctx.enter_context(tc.tile_pool(name="qkv", bufs=1))
        pr_pool = ctx.enter_context(tc.tile_pool(name="probs", bufs=6))
        acc_pool = ctx.enter_context(tc.tile_pool(name="acc", bufs=3))
        rec_pool = ctx.enter_context(tc.tile_pool(name="rec", bufs=2))
        bcs_pool = ctx.enter_context(tc.tile_pool(name="bcs", bufs=2))
        at_pool = ctx.enter_context(tc.tile_pool(name="attnT", bufs=1))
        os_pool = ctx.enter_context(tc.tile_pool(name="osb", bufs=4))
        if use_mask:
            mk_pool = ctx.enter_context(tc.tile_pool(name="mask", bufs=4))

        # PSUM: 8 banks total.
        #   sc: 2x [128,1024] (scores pairs)      -> 4 banks
        #   pv: 2x [128,512] (pv chains, also proj chains) -> 2
        #   rb: 1x [128,512] (rowsum + broadcast) -> 1
        #   op: 1x [128,512] (out-proj, also proj chains)  -> 1
        sc_ps = ctx.enter_context(tc.tile_pool(name="sc_ps", bufs=2, space="PSUM"))
        pj_ps = ctx.enter_context(tc.tile_pool(name="pj_ps", bufs=1, space="PSUM"))

        # ---- load weights / constants ----
        xt00 = xpool.tile([128, KT * TC], BF16, tag="xt", name="xt00")
        wq = wpool.tile([128, KT * HPC], BF16, tag="wqT", name="wq_t")
        wk = wpool.tile([128, KT * HPC], BF16, tag="wk", name="wk_t")
        # wq + xt00 first (the opening q chains need only those); wk after.
        # small leading pieces so the first chain starts ASAP
        for lo, hi in ((0, 2), (2, 4), (4, 8), (8, 12), (12, 16)):
            ksl = slice(lo, hi)
            nc.sync.dma_start(
                wq[:, lo * HPC:hi * HPC].rearrange("p (i j) -> p i j", j=HPC),
                wqT[ksl].rearrange("i p j -> p i j"))
            nc.sync.dma_start(
                xt00[:, lo * TC:hi * TC].rearrange("p (i j) -> p i j", j=TC),
                xT[0, 0, ksl].rearrange("i p j -> p i j"))
        nc.sync.dma_start(wk[:].rearrange("p (i j) -> p i j", j=HPC),
                          wkT.rearrange("i p j -> p i j"))
        # wv + batch-0's remaining x chunks prefetch on the scalar engine's
        # HWDGE queue so they overlap the sync queue's critical first loads
        wv = wpool.tile([128, KT * HPC], BF16, tag="wv")
        nc.gpsimd.dma_start(wv[:].rearrange("p (i j) -> p i j", j=HPC),
                            wvT.rearrange("i p j -> p i j"))
        xts = {(0, 0): xt00}
        for c in range(1, TCH):
            xt_p = xpool.tile([128, KT * TC], BF16, tag="xt", name=f"xt0{c}")
            nc.gpsimd.dma_start(xt_p[:].rearrange("p (i j) -> p i j", j=TC),
                                xT[0, c].rearrange("i p j -> p i j"))
            xts[(0, c)] = xt_p

        def prefetch_b1():
            for c in range(TCH):
                xt_p = xpool.tile([128, KT * TC], BF16, tag="xt",
                                  name=f"xt1{c}")
                nc.sync.dma_start(xt_p[:].rearrange("p (i j) -> p i j", j=TC),
                                  xT[1, c].rearrange("i p j -> p i j"))
                xts[(1, c)] = xt_p
        wo = wpool.tile([128, HEADS_PC * H], BF16, tag="wo")

        ones_col = cpool.tile([128, 1], BF16, tag="ones_col")
        nc.vector.memset(ones_col[:], 1.0)
        # memset can't write f32r directly (ISA reject): stage via f32
        ones_row_f = cpool.tile([1, 128], F32, tag="ones_row_f")
        nc.vector.memset(ones_row_f[:], 1.0)
        ones_row = cpool.tile([1, 128], F32R, tag="ones_row")
        nc.vector.tensor_copy(ones_row[:], ones_row_f[:])

        if use_bias:
            bq_t = cpool.tile([128, HEADS_PC], F32, tag="bq")
            nc.sync.dma_start(bq_t[:], bqd.rearrange("h p -> p h"))
            bk_t = cpool.tile([128, HEADS_PC], F32, tag="bk")
            nc.sync.dma_start(bk_t[:], bkd.rearrange("h p -> p h"))
            bv_row = cpool.tile([1, HPC], F32R, tag="bv_row")
            nc.sync.dma_start(bv_row[:], bvd)
            bv_ps = pj_ps.tile([128, HPC], F32, tag="rb")
            nc.tensor.matmul(bv_ps[:], r(ones_row[:]), r(bv_row[:]),
                             start=True, stop=True)
            bv_bc = cpool.tile([128, HPC], F32, tag="bv_bc")
            nc.vector.tensor_copy(bv_bc[:], bv_ps[:])

        pending = [None]  # deferred out-proj: (b, attnT, q_lo, qc)

        def emit_oproj(ob_b, ob_attnT, ob_qlo, ob_qc, final_chunk):
            for st in range(ob_qc // 128):
                tt = (ob_qlo // 128) + st
                ob = os_pool.tile([128, H], BF16, tag="ob")
                for oc in range(4):
                    # rotate psum banks so staging copies overlap the next
                    # block's matmuls; the final chunk has no later proj
                    # phase to WAR-couple with, so it also borrows the
                    # (drained) sc banks
                    blk = 4 * st + oc
                    if final_chunk and blk % 3 == 2:
                        psw = sc_ps.tile([128, 2 * QC], F32, tag="sc",
                                         name="ps_os")
                        ps = psw[:, 0:512]
                    elif blk % 2 == 1:
                        ps = pj_ps.tile([128, 512], F32,
                                        tag="pv" if final_chunk else "rb",
                                        bufs=2 if final_chunk else 1,
                                        name="ps_or")
                    else:
                        ps = pj_ps.tile([128, 512], F32, tag="op", bufs=1,
                                        name="ps_o")
                    for h in range(HEADS_PC):
                        nc.tensor.matmul(
                            ps[:],
                            r(ob_attnT[h][:, st * 128:(st + 1) * 128]),
                            r(wo[:, h * H + oc * 512: h * H + (oc + 1) * 512]),
                            start=(h == 0), stop=(h == HEADS_PC - 1))
                    if (st + oc) % 2 == 0:
                        nc.vector.tensor_copy(
                            ob[:, oc * 512:(oc + 1) * 512], ps[:])
                    else:
                        nc.scalar.copy(
                            ob[:, oc * 512:(oc + 1) * 512], ps[:])
                rows = slice(ob_b * T + tt * 128, ob_b * T + (tt + 1) * 128)
                if final_chunk and st == ob_qc // 128 - 1:
                    # split the program's very last store so its first half
                    # overlaps the second half's staging copies
                    nc.sync.dma_start(out[rows, 0:H // 2], ob[:, 0:H // 2])
                    nc.sync.dma_start(out[rows, H // 2:H], ob[:, H // 2:H])
                else:
                    nc.sync.dma_start(out[rows, :], ob[:])

        for b in range(B):
            # ---- q/k/v projections for this batch ----
            # q2/k2 hold both heads side by side: col h*T + t
            q2 = qkv_pool.tile([128, HEADS_PC * T], BF16, tag="q2", bufs=2)
            k2 = qkv_pool.tile([128, HEADS_PC * T], BF16, tag="k2", bufs=2)
            vt = qkv_pool.tile([128, KT * HPC], BF16, tag="v", bufs=2)  # [t-tile, d]

            for c in range(TCH):
                xt = xts[(b, c)]
                # q chains before k chains: the opening q chains only need
                # wq + xt, so the wk load can trail. Both heads' chains share
                # one 2-bank sc tile -> a single wide psum->sbuf copy.
                for w_, dst2, bias_t in ((wq, q2, "bq"), (wk, k2, "bk")):
                    psw = sc_ps.tile([128, 2 * QC], F32, tag="sc", name="ps_qk")
                    for i in range(KT):
                        for h in range(HEADS_PC):
                            nc.tensor.matmul(
                                psw[:, h * TC:(h + 1) * TC],
                                r(w_[:, i * HPC + 128 * h: i * HPC + 128 * h + 128]),
                                r(xt[:, i * TC: (i + 1) * TC]),
                                start=(i == 0), stop=(i == KT - 1))
                    if use_bias:
                        bt = bq_t if bias_t == "bq" else bk_t
                        for h in range(HEADS_PC):
                            nc.scalar.activation(
                                dst2[:, h * T + c * TC: h * T + (c + 1) * TC],
                                psw[:, h * TC:(h + 1) * TC],
                                AF.Identity, bias=bt[:, h:h + 1])
                    else:
                        nc.vector.tensor_copy(
                            dst2[:].rearrange("p (h t) -> p h t",
                                              h=HEADS_PC)[:, :,
                                              c * TC:(c + 1) * TC],
                            psw[:].rearrange("p (h t) -> p h t", h=HEADS_PC))
                    if c == 0 and w_ is wq and pending[0] is not None:
                        # previous batch's deferred out-proj lands here, after
                        # the first fresh q chain keeps the PE fed
                        pb, pattnT, pq_lo, pqc = pending[0]
                        pending[0] = None
                        emit_oproj(pb, pattnT, pq_lo, pqc, False)
                for s in range(4):  # four 128-row t-subtiles of this chunk
                    # NOTE: each chain gets its own bank — matmul start=True
                    # clears the whole PSUM bank on HW, so two chains must
                    # never share one bank
                    tt = 4 * c + s
                    ps = pj_ps.tile([128, TC], F32, tag="pv", bufs=2,
                                    name="ps_v")
                    for i in range(KT):
                        nc.tensor.matmul(
                            ps[:, 0:HPC],
                            r(xt[:, i * TC + 128 * s: i * TC + 128 * s + 128]),
                            r(wv[:, i * HPC: (i + 1) * HPC]),
                            start=(i == 0), stop=(i == KT - 1))
                    if use_bias:
                        nc.vector.scalar_tensor_tensor(
                            vt[:, tt * HPC:(tt + 1) * HPC], ps[:, 0:HPC], 1.0,
                            bv_bc[:], op0=OP.mult, op1=OP.add)
                    else:
                        nc.vector.tensor_copy(vt[:, tt * HPC:(tt + 1) * HPC],
                                              ps[:, 0:HPC])

            if b == 0:
                nc.sync.dma_start(wo[:].rearrange("p (i j) -> p i j", j=H),
                                  woT.rearrange("i p j -> p i j"))
                prefetch_b1()

            # ---- attention (chunk-outer) interleaved with out-proj ----
            chunks = [(c * QC, QC) for c in range(NQC)]
            for ci, (q_lo, qc) in enumerate(chunks):
                attnT = [at_pool.tile([128, QC], BF16, tag=f"a{h}",
                                      name=f"attnT{h}", bufs=2)
                         for h in range(HEADS_PC)]
                kpt = (2 * QC) // qc  # ktiles folded into one sc tile
                for h in range(HEADS_PC):
                    q_sl = r(q2[:, h * T + q_lo:h * T + q_lo + qc])
                    pv = pj_ps.tile([128, QC], F32, tag="pv", bufs=2, name="pv")
                    rs = pj_ps.tile([1, QC], F32, tag="rb", bufs=1, name="rs")
                    lvl = {}  # add-tree: level -> pending tile
                    for j in range(KT // kpt):
                        sc = sc_ps.tile([128, 2 * QC], F32, tag="sc")
                        for sub in range(kpt):
                            i = kpt * j + sub
                            nc.tensor.matmul(
                                sc[:, sub * qc:(sub + 1) * qc],
                                r(k2[:, h * T + i * 128:h * T + (i + 1) * 128]),
                                q_sl, start=True, stop=True)
                        if use_mask:
                            mk = mk_pool.tile([128, 2 * QC], F32, tag="mk")
                            for sub in range(kpt):
                                i = kpt * j + sub
                                nc.sync.dma_start(
                                    mk[:, sub * qc:(sub + 1) * qc],
                                    mkd[b, i, :, q_lo:q_lo + qc])
                            nc.vector.tensor_add(sc[:], sc[:], mk[:])
                        pr = pr_pool.tile([128, 2 * QC], BF16, tag="pr")
                        nc.scalar.activation(pr[:], sc[:], AF.Exp, scale=SCALE)
                        for sub in range(kpt):
                            i = kpt * j + sub
                            nc.tensor.matmul(
                                pv[:, 0:qc],
                                r(vt[:, i * HPC + 128 * h: i * HPC + 128 * h + 128]),
                                r(pr[:, sub * qc:(sub + 1) * qc]),
                                start=(i == 0), stop=(i == KT - 1))
                        # fold the sc tile's ktiles to one node (kpt-1 adds),
                        # then merge binary-tree style; 15 FD=512 adds per
                        # head (finer DVE ops interleave better with the
                        # latency-critical bcs/stt/copy traffic than fewer
                        # FD=1024 ops would)
                        node = acc_pool.tile([128, QC], BF16, tag="t0",
                                             name="tree0", bufs=2)
                        nc.vector.tensor_add(node[:, 0:qc], pr[:, 0:qc],
                                             pr[:, qc:2 * qc])
                        for sub in range(2, kpt):
                            nxt = acc_pool.tile([128, QC], BF16, tag="t0f",
                                                name="tree0f", bufs=4)
                            nc.vector.tensor_add(
                                nxt[:, 0:qc], node[:, 0:qc],
                                pr[:, sub * qc:(sub + 1) * qc])
                            node = nxt
                        l = 1
                        while l in lvl:
                            nxt = acc_pool.tile([128, QC], BF16, tag=f"t{l}",
                                                name=f"tree{l}")
                            nc.vector.tensor_add(nxt[:, 0:qc],
                                                 lvl.pop(l)[:, 0:qc],
                                                 node[:, 0:qc])
                            node, l = nxt, l + 1
                        lvl[l] = node
                    total = lvl[max(lvl)]
                    nc.tensor.matmul(rs[:, 0:qc], r(ones_col[:]),
                                     r(total[:, 0:qc]), start=True, stop=True)
                    rec = rec_pool.tile([1, QC], F32R, tag="rec")
                    with nc.allow_low_precision(reason="f32r rowsum reciprocal"):
                        nc.vector.reciprocal(rec[:, 0:qc], rs[:, 0:qc])
                    bc = pj_ps.tile([128, QC], F32, tag="rb", bufs=1, name="bc")
                    nc.tensor.matmul(bc[:, 0:qc], r(ones_row[:]),
                                     r(rec[:, 0:qc]), start=True, stop=True)
                    bcs = bcs_pool.tile([128, QC], F32, tag="bcs")
                    nc.vector.tensor_copy(bcs[:, 0:qc], bc[:, 0:qc])
                    nc.vector.scalar_tensor_tensor(
                        attnT[h][:, 0:qc], pv[:, 0:qc], 1.0, bcs[:, 0:qc],
                        op0=OP.mult, op1=OP.mult)

                # out-proj for this chunk's t-tiles (partial over local dims);
                # the batch's last chunk defers to the next batch's proj phase
                # so its attention drain overlaps fresh PE work
                if ci == len(chunks) - 1:
                    pending[0] = (b, attnT, q_lo, qc)
                else:
                    emit_oproj(b, attnT, q_lo, qc, False)
        pb, pattnT, pq_lo, pqc = pending[0]
        emit_oproj(pb, pattnT, pq_lo, pqc, True)

    nc.compile()
    _CACHE[key] = nc
    return nc


def prepare(inputs):
    hs = np.ascontiguousarray(np.asarray(inputs["hidden_states"], dtype=np.float32))
    mask = np.asarray(inputs["attention_mask"], dtype=np.float32)
    Wq = np.asarray(inputs["Wq"], dtype=np.float32)
    Wk = np.asarray(inputs["Wk"], dtype=np.float32)
    Wv = np.asarray(inputs["Wv"], dtype=np.float32)
    Wo = np.asarray(inputs["Wo"], dtype=np.float32)
    bq = np.asarray(inputs["bq"], dtype=np.float32)
    bk = np.asarray(inputs["bk"], dtype=np.float32)
    bv = np.asarray(inputs["bv"], dtype=np.float32)

    use_mask = bool(np.any(mask))
    use_bias = bool(np.any(bq) or np.any(bk) or np.any(bv))
    nc = _build(use_mask, use_bias)

    # x[b].T -> [h,t] -> (16,128, 4,512) -> [4,16,128,512]
    xTh = hs.transpose(0, 2, 1).reshape(B, KT, 128, TCH, TC)
    xTh = np.ascontiguousarray(xTh.transpose(0, 3, 1, 2, 4)).astype(NPBF16)
    xflat = xTh.reshape(-1)

    in_maps = []
    for c in range(N_CORES):
        sl = slice(c * HPC, (c + 1) * HPC)
        blob = np.concatenate([
            xflat,
            np.ascontiguousarray(Wq[sl].T).astype(NPBF16).reshape(-1),
            np.ascontiguousarray(Wk[sl].T).astype(NPBF16).reshape(-1),
            np.ascontiguousarray(Wv[sl].T).astype(NPBF16).reshape(-1),
            np.ascontiguousarray(Wo[:, sl].T).astype(NPBF16).reshape(-1),
        ])
        assert blob.shape == (BLOB_SZ,)
        m = {"blob": blob}
        if use_bias:
            m["bq"] = np.ascontiguousarray(bq[sl]).reshape(HEADS_PC, 128)
            m["bk"] = np.ascontiguousarray(bk[sl]).reshape(HEADS_PC, 128)
            m["bv"] = np.ascontiguousarray(bv[sl]).reshape(1, HPC)
        if use_mask:
            mt = mask[:, 0].transpose(0, 2, 1) / SCALE  # [B, tk, tq]
            m["maskT"] = np.ascontiguousarray(mt).reshape(B, KT, 128, T)
        in_maps.append(m)
    return nc, in_maps


def postprocess(results, inputs):
    bo = np.asarray(inputs["bo"], dtype=np.float32)
    acc = results[0]["out"].astype(np.float32)
    for c in range(1, N_CORES):
        acc = acc + results[c]["out"].astype(np.float32)
    return (acc + bo).reshape(B, T, H)


def kernel(**inputs):
    import time as _time

    nc, in_maps = prepare(inputs)
    last_err = None
    for attempt in range(3):
        try:
            res = run_bass_kernel_spmd(nc, in_maps, list(range(N_CORES)))
            result = postprocess(res.results, inputs)
            if np.isfinite(result).all():
                return result
            # transient device fault can yield garbage without raising
            last_err = ValueError("non-finite kernel output")
        except Exception as e:
            last_err = e
        _time.sleep(2.0)
        try:  # best-effort device recovery before retrying
            import jax
            jax.extend.backend.clear_backends()
        except Exception:
            pass
    raise last_err
